# revision 5
# baseline (speedup 1.0000x reference)
"""Trainium2 Bass kernel for nn_DTA_36481452212801.

Reference computation (per batch element b of 8, one NeuronCore each):
    k0 = l2norm_n(adaptive_max_pool_c(x1, 130))        # (n=4096, K=130)
    q0 = l2norm_n(adaptive_max_pool_c(x2, 130))
    3 stages of (per stream s in {x1->k, x2->q}):
        z  = x^T s          # (c=512, K)  contract n
        z  = softmax_K(z)
        z  = l2norm_c(z)
        s  = x z            # (n, K)      contract c
        s  = l2norm_n(s)
    returns (k, q), each (8, 4096, 130) fp32

Sharding: pure data parallel, batch element i -> core i (8 cores).

Kernel design (per core):
  - x kept in SBUF in two layouts: x [128, 32, 512] (n-partitioned) for the
    z-matmul (contraction over n) and xT [128, 4, 4096] (c-partitioned,
    built with PE transposes) for the s-matmul (contraction over c).
  - s stored [128, 32, KS] (n-partitioned blocks along free dim).
  - l2norm column scales are folded into the *next* stage's z (z columns get
    scaled by the pending s-colscale right after the z-matmul), so the big
    [4096, K] scale multiply happens only once per stream at the very end.
  - column sums (over partitions+blocks) via PE matmul with a ones vector;
    partition broadcast of [1, K] vectors via PE outer product with ones.
  - adaptive max pool: windows are length 4/5 with starts (i*512)//130; for
    each shift j the gather columns decompose into ~9 constant-stride runs,
    done as strided DVE copies/max-accumulates.
"""

import sys

sys.path.insert(0, "/opt/trn_rl_repo")

import numpy as np

import concourse.bass as bass
import concourse.tile as tile
from concourse import mybir
from concourse.bass_utils import run_bass_kernel_spmd
from concourse.masks import make_identity

from waitsplit import split_excess_waits  # noqa: E402  (same dir at build time)

F32 = mybir.dt.float32
ALU = mybir.AluOpType
AFT = mybir.ActivationFunctionType

B, N, C, K = 8, 4096, 512, 130
P = 128
NB = N // P  # 32 n-blocks
CB = C // P  # 4 c-chunks
STAGES = 3
KS = 130  # free-dim stride for K columns in s tiles (pad to 256 for fp32r)
MM_DT = F32  # matmul operand dtype view

EPS = 1e-6


def _pool_runs():
    """Adaptive-max-pool gather plan: list of (j, runs); each run is
    (i0, cnt, col0, step): pooled[:, i0+t] accumulates x[:, col0+step*t]."""
    i = np.arange(K)
    starts = (i * C) // K
    ends = -((-(i + 1) * C) // K)
    wins = ends - starts
    plans = []
    for j in range(int(wins.max())):
        sel = np.where(wins > j)[0]
        cols = starts[sel] + j
        runs = []
        t = 0
        while t < len(sel):
            u = t
            if t + 1 < len(sel) and sel[t + 1] == sel[t] + 1:
                st = int(cols[t + 1] - cols[t])
                while (
                    u + 1 < len(sel)
                    and sel[u + 1] == sel[u] + 1
                    and int(cols[u + 1] - cols[u]) == st
                ):
                    u += 1
            else:
                st = 1
            runs.append((int(sel[t]), u - t + 1, int(cols[t]), st))
            t = u + 1
        plans.append((j, runs))
    return plans


def _strided_cols(tile3d, c0, cnt, step):
    """AP selecting columns c0 + step*t (t<cnt) of a [128, NB, W] tile."""
    base = tile3d[:, :, c0 : c0 + 1]
    ap = [list(d) for d in base.ap]
    ap[-1] = [step * ap[-1][0] if ap[-1][0] != 0 else step, cnt]
    return bass.AP(tensor=base.tensor, offset=base.offset, ap=ap)


def _bcast_mid(ap2d, times):
    """[P, K] AP -> [P, times, K] with a step-0 middle dim."""
    a = ap2d[:, :]
    return bass.AP(
        tensor=a.tensor,
        offset=a.offset,
        ap=[list(a.ap[0]), [0, times], list(a.ap[1])],
    )


def _bcast_last(ap2d, times):
    """[P, M] AP -> [P, M, times] with a step-0 innermost dim."""
    a = ap2d[:, :]
    return bass.AP(
        tensor=a.tensor,
        offset=a.offset,
        ap=[list(a.ap[0]), list(a.ap[1]), [0, times]],
    )


def build_nc(split=True):
    nc = bass.Bass()
    x_in = [
        nc.declare_dram_parameter("x1", [N, C], F32, isOutput=False),
        nc.declare_dram_parameter("x2", [N, C], F32, isOutput=False),
    ]
    y_out = [
        nc.declare_dram_parameter("k", [N, K], F32, isOutput=True),
        nc.declare_dram_parameter("q", [N, K], F32, isOutput=True),
    ]

    pool_plan = _pool_runs()

    with tile.TileContext(nc) as tc:
        from contextlib import ExitStack

        with ExitStack() as ctx:
            px = ctx.enter_context(tc.tile_pool(name="px", bufs=1))
            pxt = ctx.enter_context(tc.tile_pool(name="pxt", bufs=1))
            ps = ctx.enter_context(tc.tile_pool(name="ps", bufs=1))
            psq = ctx.enter_context(tc.tile_pool(name="psq", bufs=1))
            pzp = ctx.enter_context(tc.tile_pool(name="pzp", bufs=2))
            psm = ctx.enter_context(tc.tile_pool(name="psm", bufs=2))
            ptiny = ctx.enter_context(tc.tile_pool(name="ptiny", bufs=4))
            pconst = ctx.enter_context(tc.tile_pool(name="pconst", bufs=1))
            ppz = ctx.enter_context(tc.tile_pool(name="ppz", bufs=1, space="PSUM"))
            ppk = ctx.enter_context(tc.tile_pool(name="ppk", bufs=2, space="PSUM"))

            # constants
            ident = pconst.tile([P, P], MM_DT, tag="ident")
            make_identity(nc, ident)
            ones_col = pconst.tile([P, 1], MM_DT, tag="ones_col")
            nc.vector.memset(ones_col, 1.0)
            ones_row = pconst.tile([1, P], MM_DT, tag="ones_row")
            nc.vector.memset(ones_row, 1.0)

            def colscale_chain(cs_ps, tag):
                """cs_ps [1,K] psum (sum of squares per column) ->
                bc [P,K] psum broadcast of 1/(eps+sqrt(cs))."""
                nrm = ptiny.tile([1, K], F32, tag="nrm")
                nc.scalar.activation(out=nrm, in_=cs_ps, func=AFT.Sqrt)
                nc.vector.tensor_scalar_add(out=nrm, in0=nrm, scalar1=EPS)
                rcol = ptiny.tile([1, K], F32, tag="rcol")
                nc.vector.reciprocal(out=rcol, in_=nrm)
                bc_ps = ppk.tile([P, K], F32, tag="kn")
                nc.tensor.matmul(
                    bc_ps, lhsT=ones_row, rhs=rcol, start=True, stop=True
                )
                bc = ptiny.tile([P, K], F32, tag="bc")
                nc.scalar.copy(out=bc, in_=bc_ps)
                return bc

            for si in range(2):
                # ---- load x ----
                x_t = px.tile([P, NB, C], F32, tag="x")
                xa = x_in[si][:, :].rearrange("(b p) c -> p b c", p=P)
                for g in range(4):
                    nc.sync.dma_start(
                        out=x_t[:, 8 * g : 8 * (g + 1), :],
                        in_=xa[:, 8 * g : 8 * (g + 1), :],
                    )

                # ---- transpose x -> xT ----
                xT_t = pxt.tile([P, CB, N], F32, tag="xT")
                for ct in range(CB):
                    for g in range(4):  # 8 b-blocks per psum fill
                        tp = ppk.tile([P, 2, 512], F32, tag="kn")
                        for jj in range(2):
                            for r in range(4):
                                b = 8 * g + 4 * jj + r
                                nc.tensor.transpose(
                                    tp[:, jj, 128 * r : 128 * (r + 1)],
                                    x_t[:, b, 128 * ct : 128 * (ct + 1)],
                                    ident,
                                )
                        eng = nc.scalar if g % 2 == 0 else nc.vector
                        dst = xT_t[:, ct, 1024 * g : 1024 * (g + 1)]
                        src = bass.AP(
                            tensor=tp.tensor,
                            offset=tp[:, 0, 0:1].offset,
                            ap=[list(tp[:, 0, 0:1].ap[0]), [1, 1024]],
                        )
                        if g % 2 == 0:
                            nc.scalar.copy(out=dst, in_=src)
                        else:
                            nc.vector.tensor_copy(out=dst, in_=src)

                # ---- adaptive max pool into s ----
                s_t = ps.tile([P, NB, KS], F32, tag="s")
                for j, runs in pool_plan:
                    for (i0, cnt, c0, st) in runs:
                        dst = s_t[:, :, i0 : i0 + cnt]
                        src = _strided_cols(x_t, c0, cnt, st)
                        if j == 0:
                            nc.vector.tensor_copy(out=dst, in_=src)
                        else:
                            nc.vector.tensor_tensor(
                                out=dst, in0=dst, in1=src, op=ALU.max
                            )

                # ---- init colscale (of pooled s) ----
                sq_t = psq.tile([P, NB, KS], F32, tag="sq")
                nc.scalar.activation(
                    out=sq_t[:, :, 0:K], in_=s_t[:, :, 0:K], func=AFT.Square
                )
                cs = ppk.tile([1, K], F32, tag="kn")
                for b in range(NB):
                    nc.tensor.matmul(
                        cs,
                        lhsT=ones_col,
                        rhs=sq_t[:, b, 0:K],
                        start=(b == 0),
                        stop=(b == NB - 1),
                    )
                bc_s = colscale_chain(cs, "init")

                # ---- stages ----
                for stage in range(STAGES):
                    # z = x^T s  (contract n; fold pending s colscale in)
                    z_ps = ppz.tile([P, CB, 512], F32, tag="z")
                    for ch in range(CB):
                        for b in range(NB):
                            nc.tensor.matmul(
                                z_ps[:, ch, 0:K],
                                lhsT=x_t[:, b, 128 * ch : 128 * (ch + 1)],
                                rhs=s_t[:, b, 0:K],
                                start=(b == 0),
                                stop=(b == NB - 1),
                            )
                    # zt = z * bc_s (column scale from pending l2norm of s)
                    zt = psm.tile([P, CB, K], F32, tag="zb")
                    nc.vector.tensor_tensor(
                        out=zt,
                        in0=z_ps[:, :, 0:K],
                        in1=_bcast_mid(bc_s, CB),
                        op=ALU.mult,
                    )
                    # softmax over K (free dim)
                    mx = ptiny.tile([P, CB], F32, tag="mx")
                    nc.vector.reduce_max(out=mx, in_=zt, axis=mybir.AxisListType.X)
                    nc.vector.tensor_tensor(
                        out=zt, in0=zt, in1=_bcast_last(mx, K), op=ALU.subtract
                    )
                    et = psm.tile([P, CB, K], F32, tag="eb")
                    sums = ptiny.tile([P, CB], F32, tag="sums")
                    for ch in range(CB):
                        nc.scalar.activation(
                            out=et[:, ch, :],
                            in_=zt[:, ch, :],
                            func=AFT.Exp,
                            accum_out=sums[:, ch : ch + 1],
                        )
                    rs = ptiny.tile([P, CB], F32, tag="rs")
                    nc.vector.reciprocal(out=rs, in_=sums)
                    for ch in range(CB):
                        nc.vector.tensor_scalar_mul(
                            out=et[:, ch, :],
                            in0=et[:, ch, :],
                            scalar1=rs[:, ch : ch + 1],
                        )
                    # l2norm over c of softmaxed z -> z'
                    nc.scalar.activation(out=zt, in_=et, func=AFT.Square)
                    cs2 = ppk.tile([1, K], F32, tag="kn")
                    for ch in range(CB):
                        nc.tensor.matmul(
                            cs2,
                            lhsT=ones_col,
                            rhs=zt[:, ch, :],
                            start=(ch == 0),
                            stop=(ch == CB - 1),
                        )
                    bc_z = colscale_chain(cs2, f"z{stage}")
                    zp = pzp.tile([P, CB, KS], F32, tag="zp")
                    nc.vector.tensor_tensor(
                        out=zp[:, :, 0:K],
                        in0=et,
                        in1=_bcast_mid(bc_z, CB),
                        op=ALU.mult,
                    )
                    # s' = x z'  (contract c) + drain psum -> s
                    for g in range(NB // 2):
                        kn = ppk.tile([P, 2, 512], F32, tag="kn")
                        for jj in range(2):
                            m = 2 * g + jj
                            for ct in range(CB):
                                nc.tensor.matmul(
                                    kn[:, jj, 0:K],
                                    lhsT=xT_t[:, ct, 128 * m : 128 * (m + 1)],
                                    rhs=zp[:, ct, 0:K],
                                    start=(ct == 0),
                                    stop=(ct == CB - 1),
                                )
                        if g % 2 == 0:
                            nc.scalar.copy(
                                out=s_t[:, 2 * g : 2 * g + 2, 0:K],
                                in_=kn[:, :, 0:K],
                            )
                        else:
                            nc.vector.tensor_copy(
                                out=s_t[:, 2 * g : 2 * g + 2, 0:K],
                                in_=kn[:, :, 0:K],
                            )
                    # column sums of s'^2 for l2norm over n
                    nc.scalar.activation(
                        out=sq_t[:, :, 0:K], in_=s_t[:, :, 0:K], func=AFT.Square
                    )
                    cs3 = ppk.tile([1, K], F32, tag="kn")
                    for b in range(NB):
                        nc.tensor.matmul(
                            cs3,
                            lhsT=ones_col,
                            rhs=sq_t[:, b, 0:K],
                            start=(b == 0),
                            stop=(b == NB - 1),
                        )
                    bc_s = colscale_chain(cs3, f"s{stage}")

                # ---- final scale + store ----
                ko = sq_t  # reuse
                nc.vector.tensor_tensor(
                    out=ko[:, :, 0:K],
                    in0=s_t[:, :, 0:K],
                    in1=_bcast_mid(bc_s, NB),
                    op=ALU.mult,
                )
                ya = y_out[si][:, :].rearrange("(b p) i -> p b i", p=P)
                for g in range(4):
                    nc.sync.dma_start(
                        out=ya[:, 8 * g : 8 * (g + 1), :],
                        in_=ko[:, 8 * g : 8 * (g + 1), 0:K],
                    )

    if split:
        split_excess_waits(nc)
    return nc


_NC = None


def _get_nc():
    global _NC
    if _NC is None:
        _NC = build_nc()
    return _NC


TRACE = False
LAST_RESULT = None


def kernel(x1, x2):
    global LAST_RESULT
    x1 = np.ascontiguousarray(np.asarray(x1, dtype=np.float32))
    x2 = np.ascontiguousarray(np.asarray(x2, dtype=np.float32))
    assert x1.shape == (B, N, C) and x2.shape == (B, N, C)
    nc = _get_nc()
    in_maps = [{"x1": x1[i], "x2": x2[i]} for i in range(B)]
    r = run_bass_kernel_spmd(nc, in_maps, list(range(B)), trace=TRACE)
    LAST_RESULT = r
    k = np.stack([r.results[i]["k"] for i in range(B)])
    q = np.stack([r.results[i]["q"] for i in range(B)])
    return (k, q)
